# revision 62
# baseline (speedup 1.0000x reference)
"""Trainium2 Bass kernel for nn_GAttention (gnn_message_passing).

Reference computation (per batch b):
    q = s[:,b,:] @ Qweight                      # (N, H)
    k = Kweight.T @ s[:,b,:]                    # (H, I)   (contraction over n)
    att1 = (q @ k) * (1/sqrt(H)) + 1e-9         # (N, I)
    att2 = att1**2 @ Gmat                       # (N, I)
    out[:,b,:] = att2 / (rowsum(att2) + 1e-3)

Sharding: pure data-parallel over batch B=16 -> 2 batches per core on 8 cores.
Gmat/Qweight/Kweight replicated.

v4: bf16 front-end + fp8 DoubleRow back-end.
  - att1sq and Gmat are fp8e4 and the dominant att2 GEMM (1.07 GMAC/batch)
    runs double-pumped (MatmulPerfMode.DoubleRow, 2 contraction chunks per
    instruction, 216ns per 512-wide matmul == 157 TF/s). Measured end-to-end
    rel err ~2.7e-3 vs the f32 reference (gate 2e-2): fp8 noise feeds
    all-positive 1024-term sums and the normalization rowsum is accumulated
    from the same quantized products, so num/denom stay consistent.
  - everything upstream (s, s_T, k, q, att1) stays bf16: fp8 CONVERSIONS on
    ACT/DVE measure ~2x slower than bf16 ones (f32->fp8 cast ~1.3us/chunk vs
    0.73, PSUM->fp8 evict 710ns vs 418ns per 512), so fp8 there loses more
    engine time than DoubleRow saves in PE time.
  - normalize: the two att2 halves accumulate into one 2-bank PSUM tile so
    a single 1024-wide ACT eviction with fused accum_out yields the full
    rowsum in one op; rinv is a tiny DVE add+reciprocal and one 1024-wide
    DVE multiply scales the tile in place; each output half DMAs as soon as
    it is scaled. (GPSIMD was tried for the scale: 14.8us per op, ~30x
    slower than DVE, plus SBUF port contention.)
  - HBM is the binding roofline (20.8 MB/core, 358 GB/s peak; outputs
    sustain only ~200 GB/s on 4KB-row strided writes). All input DMAs ride
    the Sync queue set in priority order s(b0) -> s(b1) -> G with NO
    completion-dependency ladders (those serialized input arrival to ~59us).
  - engine-queue discipline: each engine runs its ops in emission order, so
    ops feeding the PE next are never emitted behind ops gated on later
    results (ALL G casts deferred past batch0's A/B phases, kq evictions
    split ACT/DVE to halve their serial latency).
  - lead-in: batch0 chunk0 arrives as four column-quarter DMAs on separate
    queues and is cast/transposed piecewise, so the PE starts ~10us in.
  - att2 group nt only reads att1sq n-columns nt*128:(nt+1)*128 (half
    nt//4), so only half-0 att1 tiles gate each C phase; the others are
    woven into the att2 group streams.

The two batches are software-pipelined: batch1's cast/transpose chunks are
interleaved with batch0's att1 tiles (trailing the s1 DMA stream), its k/q
are built just before C(0), and its att1 tiles are woven into both att2
group streams so the PE always has independent work while PSUM banks drain.
Run-to-run HW variance is ~+/-10% (108-127us observed for this binary).
"""

import sys

import numpy as np

try:  # concourse normally comes from the image's NIX_PYTHONPATH
    import concourse  # noqa: F401
except ImportError:  # pragma: no cover
    sys.path.insert(0, "/opt/trn_rl_repo")

N_DIM = 1024
IN_DIM = 1024
H_DIM = 64
B = 16
N_CORES = 8
B_LOC = B // N_CORES  # batches per core

P = 128          # SBUF/PSUM partitions
NCH_N = N_DIM // P   # 8 chunks over n
NCH_I = IN_DIM // P  # 8 chunks over i
NH = 512         # psum free-dim half (one fp32 bank)
NPAIR = NCH_I // 2   # DoubleRow contracts chunk pairs

_NC_CACHE = {}


def _build_nc(mm_mode="v4"):
    import concourse.bass as bass
    import concourse.tile as tile
    from concourse import bacc, mybir
    from concourse.masks import make_identity

    f32 = mybir.dt.float32
    bf16 = mybir.dt.bfloat16
    fp8 = mybir.dt.float8e4
    AFT = mybir.ActivationFunctionType
    ALU = mybir.AluOpType
    DR = mybir.MatmulPerfMode.DoubleRow

    nc = bacc.Bacc(
        "TRN2",
        target_bir_lowering=False,
        debug=False,
        num_devices=N_CORES,
    )
    s_d = nc.dram_tensor("s", [N_DIM, B_LOC, IN_DIM], f32, kind="ExternalInput")
    g_d = nc.dram_tensor("gmat", [IN_DIM, IN_DIM], f32, kind="ExternalInput")
    qw_d = nc.dram_tensor("qw", [IN_DIM, H_DIM], f32, kind="ExternalInput")
    kw_d = nc.dram_tensor("kw", [N_DIM, H_DIM], f32, kind="ExternalInput")
    o_d = nc.dram_tensor("out", [N_DIM, B_LOC, IN_DIM], f32, kind="ExternalOutput")

    with tile.TileContext(nc) as tc:
        with (
            tc.tile_pool(name="const", bufs=1) as const_pool,
            tc.tile_pool(name="gmat", bufs=1) as gmat_pool,
            tc.tile_pool(name="snat", bufs=1) as snat_pool,
            tc.tile_pool(name="sT", bufs=2) as sT_pool,
            tc.tile_pool(name="att1", bufs=2) as att1_pool,
            tc.tile_pool(name="kq", bufs=1) as kq_pool,
            tc.tile_pool(name="outs", bufs=3) as out_pool,
            tc.tile_pool(name="stage", bufs=2) as stage_pool,
            tc.tile_pool(name="sbf", bufs=2) as sbf_pool,
            tc.tile_pool(name="stat", bufs=8) as stat_pool,
            tc.tile_pool(name="psA", bufs=2, space="PSUM") as psA,
            tc.tile_pool(name="psO", bufs=2, space="PSUM") as psO,
            tc.tile_pool(name="psKQ", bufs=1, space="PSUM") as psKQ,
        ):
            ident_f32 = const_pool.tile([P, P], f32)
            make_identity(nc, ident_f32[:])
            ident_bf = const_pool.tile([P, P], bf16)
            nc.vector.tensor_copy(ident_bf[:], ident_f32[:])

            eps_bias = const_pool.tile([P, 1], f32)
            nc.vector.memset(eps_bias[:], 1e-9)

            # ---- batch0 s chunks first: chunk0 split into column quarters
            # on separate queues so the first cast/transpose starts early.
            def phase_load_s(b, split0):
                s_view = s_d.ap()[:, b, :]
                s_nat = snat_pool.tile([P, NCH_N, IN_DIM], f32, tag="snat")
                dmas = []
                qtrs = None
                for cn in range(NCH_N):
                    if cn == 0 and split0:
                        qtrs = []
                        for qtr in range(4):
                            c0, c1 = qtr * 256, (qtr + 1) * 256
                            dd = nc.sync.dma_start(
                                s_nat[:, 0, c0:c1], s_view[0:P, c0:c1]
                            )
                            qtrs.append(dd)
                        dmas.append(qtrs[-1])
                    else:
                        dd = nc.sync.dma_start(
                            s_nat[:, cn, :], s_view[cn * P:(cn + 1) * P, :]
                        )
                        dmas.append(dd)
                return s_nat, dmas, qtrs

            s_nat0, s_dmas0, s0_qtrs = phase_load_s(0, split0=True)

            # weights (small) after the critical s chunks
            qw_f32 = const_pool.tile([P, NCH_I, H_DIM], f32)
            nc.sync.dma_start(
                qw_f32[:], qw_d.ap().rearrange("(c p) h -> p c h", p=P)
            )
            qw_sb = const_pool.tile([P, NCH_I, H_DIM], bf16)
            nc.vector.tensor_copy(qw_sb[:], qw_f32[:])
            kw_f32 = const_pool.tile([P, NCH_N, H_DIM], f32)
            nc.sync.dma_start(
                kw_f32[:], kw_d.ap().rearrange("(c p) h -> p c h", p=P)
            )
            kw_sb = const_pool.tile([P, NCH_N, H_DIM], bf16)
            nc.vector.tensor_copy(kw_sb[:], kw_f32[:])

            # HBM priority (one Sync queue set, order = priority, no
            # completion ladders): s(b0) -> s(b1) -> G. Batch1's
            # cast/transpose work is s1-arrival-paced and overlaps batch0's
            # B phase; G is needed last (first att2 group). (G-before-s1 was
            # measured too: it starts C(0) ~3us earlier but stalls batch1's
            # pipeline more than it gains.)
            # Gmat staged whole in f32; ALL casts deferred -- inline casts at
            # the head of the ACT/DVE queues would block every s cast behind
            # them while waiting for G chunks to arrive.
            s_nat1, s_dmas1, _ = phase_load_s(1, split0=False)
            g_sb = gmat_pool.tile([P, NCH_I, IN_DIM], fp8)
            g_view = g_d.ap()
            g_f32 = gmat_pool.tile([P, NCH_I, IN_DIM], f32)
            for ci in range(NCH_I):
                nc.sync.dma_start(
                    g_f32[:, ci, :], g_view[ci * P:(ci + 1) * P, :]
                )

            def emit_g_casts():
                for ci in range(NCH_I):
                    if ci % 2 == 0:
                        nc.scalar.activation(
                            g_sb[:, ci, :], g_f32[:, ci, :], AFT.Copy
                        )
                    else:
                        nc.vector.tensor_copy(
                            g_sb[:, ci, :], g_f32[:, ci, :]
                        )

            def cast_chunk(s_nat, s_bf, cn, qtr=None, on_dve=False):
                # batch0 casts on ACT (free during A(0)); batch1 casts on
                # DVE -- the ACT queue at that time holds the kq evictions
                # and att1 squares, which wait on q0 and would head-of-line
                # block casts whose data is already resident.
                if qtr is None:
                    if on_dve:
                        nc.vector.tensor_copy(s_bf[:, cn, :], s_nat[:, cn, :])
                    else:
                        nc.scalar.activation(
                            s_bf[:, cn, :], s_nat[:, cn, :], AFT.Copy
                        )
                else:
                    c0, c1 = qtr * 256, (qtr + 1) * 256
                    if qtr % 2 == 0:
                        nc.scalar.activation(
                            s_bf[:, cn, c0:c1], s_nat[:, cn, c0:c1], AFT.Copy
                        )
                    else:
                        nc.vector.tensor_copy(
                            s_bf[:, cn, c0:c1], s_nat[:, cn, c0:c1]
                        )

            def transposes_cig(s_bf, s_T, cn, cig):
                """4 transpose blocks (128x128), transpose-mode so the PSUM
                tile stays bf16 and the eviction is a cheap same-dtype COPY
                (418 vs 690 ns per 512)."""
                pt = psA.tile([P, NH], bf16, tag="ps512")
                for blk in range(4):
                    ci = cig * 4 + blk
                    nc.tensor.transpose(
                        pt[:, blk * P:(blk + 1) * P],
                        s_bf[:, cn, ci * P:(ci + 1) * P],
                        ident_bf[:],
                    )
                nc.vector.tensor_copy(
                    s_T[:, cig * 4:(cig + 1) * 4, cn * P:(cn + 1) * P],
                    pt[:].rearrange("p (c n) -> p c n", c=4),
                )

            def phase_tk_chunk(s_nat, s_bf, s_T, ps_k, cn, first_split=False,
                               on_dve=False):
                """Cast + transposes + k-matmul contribution for one chunk."""
                if cn == 0 and first_split:
                    cast_chunk(s_nat, s_bf, 0, qtr=0)
                    cast_chunk(s_nat, s_bf, 0, qtr=1)
                    transposes_cig(s_bf, s_T, 0, 0)
                    cast_chunk(s_nat, s_bf, 0, qtr=2)
                    cast_chunk(s_nat, s_bf, 0, qtr=3)
                    transposes_cig(s_bf, s_T, 0, 1)
                else:
                    cast_chunk(s_nat, s_bf, cn, on_dve=on_dve)
                    for cig in range(2):
                        transposes_cig(s_bf, s_T, cn, cig)
                for half in range(2):
                    nc.tensor.matmul(
                        ps_k[:, half * NH:(half + 1) * NH],
                        kw_sb[:, cn, :],
                        s_bf[:, cn, half * NH:(half + 1) * NH],
                        start=(cn == 0),
                        stop=(cn == NCH_N - 1),
                    )

            def emit_k_evict(ps_k):
                # halves evicted on ACT and DVE concurrently: this sits on
                # the serial chain k-psum -> att1, so latency matters
                k_sb = kq_pool.tile([H_DIM, IN_DIM], bf16, tag="k")
                nc.scalar.activation(k_sb[:, 0:NH], ps_k[:, 0:NH], AFT.Copy)
                nc.vector.tensor_copy(k_sb[:, NH:2 * NH], ps_k[:, NH:2 * NH])
                return k_sb

            def emit_q(s_T):
                ps_q = psKQ.tile([H_DIM, N_DIM], f32, tag="kq")
                for ci in range(NCH_I):
                    for half in range(2):
                        nc.tensor.matmul(
                            ps_q[:, half * NH:(half + 1) * NH],
                            qw_sb[:, ci, :],
                            s_T[:, ci, half * NH:(half + 1) * NH],
                            start=(ci == 0),
                            stop=(ci == NCH_I - 1),
                        )
                q_sb = kq_pool.tile([H_DIM, N_DIM], bf16, tag="q")
                nc.scalar.activation(q_sb[:, 0:NH], ps_q[:, 0:NH], AFT.Copy)
                nc.vector.tensor_copy(q_sb[:, NH:2 * NH], ps_q[:, NH:2 * NH])
                return q_sb

            def emit_att1_group(att1sq, k_sb, q_sb, ci, half, idx):
                """att1T tile (ci, half): K=64 bf16 matmul then fused
                square+scale into fp8. Mostly ACT; some tiles on DVE."""
                pa = psA.tile([P, NH], f32, tag="ps512")
                nc.tensor.matmul(
                    pa[:],
                    k_sb[:, ci * P:(ci + 1) * P],
                    q_sb[:, half * NH:(half + 1) * NH],
                    start=True,
                    stop=True,
                )
                dst = att1sq[:, ci, half * NH:(half + 1) * NH]
                if idx % 3 != 2:
                    nc.scalar.activation(
                        dst, pa[:], AFT.Square, bias=eps_bias[:], scale=0.125
                    )
                else:
                    # DVE path needs 2 ops (hw allows only one PSUM operand);
                    # bf16 tmp gets 2x DVE rate on the second op.
                    tmp = stage_pool.tile([P, NH], bf16, tag="sqtmp")
                    nc.vector.tensor_scalar(
                        tmp[:], pa[:], 0.125, 1e-9,
                        op0=ALU.mult, op1=ALU.add,
                    )
                    nc.vector.tensor_mul(dst, tmp[:], tmp[:])

            def att2_matmuls(att1sq, nt):
                po = psO.tile([P, 2 * NH], f32, tag="psO")
                for c in range(NPAIR):
                    lhsT = att1sq[:, 2 * c:2 * c + 2, nt * P:(nt + 1) * P]
                    nc.tensor.matmul(
                        po[:, 0:NH], lhsT, g_sb[:, 2 * c:2 * c + 2, 0:NH],
                        start=(c == 0), stop=(c == NPAIR - 1),
                        perf_mode=DR,
                    )
                    nc.tensor.matmul(
                        po[:, NH:2 * NH], lhsT, g_sb[:, 2 * c:2 * c + 2, NH:2 * NH],
                        start=(c == 0), stop=(c == NPAIR - 1),
                        perf_mode=DR,
                    )
                return po

            def phase_att2_pair(att1sq0, att1sq1, nt):
                """Tile nt for BOTH batches: since the output layout is
                [N, B_LOC, IN], the paired tile [128, 2, 1024] is one fully
                CONTIGUOUS 1MB DRAM region (8KB rows), roughly doubling the
                write throughput of the stream that closes the schedule.
                Each batch's 512-halves accumulate into one 2-bank PSUM tile
                so a single 1024-wide eviction with fused accum_out yields
                that batch's full rowsum in one op (ACT for b0, DVE for b1,
                so the pair's evictions run concurrently)."""
                po0 = att2_matmuls(att1sq0, nt)
                po1 = att2_matmuls(att1sq1, nt)
                otp = out_pool.tile([P, B_LOC, IN_DIM], f32, tag="out")
                rs0 = stat_pool.tile([P, 1], f32, tag="rs0")
                rs1 = stat_pool.tile([P, 1], f32, tag="rs1")
                nc.scalar.activation(
                    otp[:, 0, :], po0[:], AFT.Copy, accum_out=rs0[:]
                )
                nc.vector.tensor_scalar(
                    otp[:, 1, :], po1[:], 0.0, 0.0,
                    op0=ALU.add, op1=ALU.add, accum_out=rs1[:],
                )
                rinv0 = stat_pool.tile([P, 1], f32, tag="rinv0")
                rinv1 = stat_pool.tile([P, 1], f32, tag="rinv1")
                nc.vector.tensor_scalar_add(rinv0[:], rs0[:], 1e-3)
                nc.vector.reciprocal(rinv0[:], rinv0[:])
                nc.vector.tensor_scalar_add(rinv1[:], rs1[:], 1e-3)
                nc.vector.reciprocal(rinv1[:], rinv1[:])
                nc.scalar.activation(
                    otp[:, 0, :], otp[:, 0, :], AFT.Copy, scale=rinv0[:]
                )
                nc.vector.tensor_scalar_mul(
                    otp[:, 1, :], otp[:, 1, :], rinv1[:]
                )
                return nc.sync.dma_start(
                    o_d.ap()[nt * P:(nt + 1) * P, :, :], otp[:]
                )

            # ---- software pipeline over the two batches. HBM is the
            # binding roofline, so the DMA stream (s0, s1, G, outputs) is
            # front-loaded and compute trails it:
            #   A(0): per-chunk cast+transpose+k as s0 chunks land
            #   kq(0), then att1(0)-half0 INTERLEAVED with batch1's
            #   cast/transpose chunks (which trail the s1 DMA stream),
            #   att1(0)-half1, k(1), G casts, q(1),
            #   C(0) weaving att1(1)-half0, C(1) weaving att1(1)-half1.
            ATT1_ORDER = [(ci, half) for half in range(2) for ci in range(NCH_I)]

            s_bf0 = sbf_pool.tile([P, NCH_N, IN_DIM], bf16, tag="sbf")
            s_T0 = sT_pool.tile([P, NCH_I, N_DIM], bf16, tag="sT")
            ps_k0 = psKQ.tile([H_DIM, IN_DIM], f32, tag="kq")
            for cn in range(NCH_N):
                phase_tk_chunk(s_nat0, s_bf0, s_T0, ps_k0, cn,
                               first_split=(cn == 0))

            k_sb0 = emit_k_evict(ps_k0)
            q_sb0 = emit_q(s_T0)

            s_bf1 = sbf_pool.tile([P, NCH_N, IN_DIM], bf16, tag="sbf")
            s_T1 = sT_pool.tile([P, NCH_I, N_DIM], bf16, tag="sT")
            att1sq0 = att1_pool.tile([P, NCH_I, N_DIM], fp8, tag="att1")
            att1sq1 = att1_pool.tile([P, NCH_I, N_DIM], fp8, tag="att1")

            # batch0's att1 half-0 tiles interleaved with batch1's
            # cast/transpose chunks (which trail the s1 DMA stream), then
            # batch0's half-1 tiles.
            for idx in range(8):
                ci, half = ATT1_ORDER[idx]
                emit_att1_group(att1sq0, k_sb0, q_sb0, ci, half, idx)
                # batch1 casts on DVE: on ACT they would queue behind the
                # q0-gated att1 squares even though their data is resident
                cast_chunk(s_nat1, s_bf1, idx, on_dve=True)
                for cig in range(2):
                    transposes_cig(s_bf1, s_T1, idx, cig)
            for idx in range(8, 16):
                ci, half = ATT1_ORDER[idx]
                emit_att1_group(att1sq0, k_sb0, q_sb0, ci, half, idx)

            # batch1 k: all chunks are resident by now, accumulate en bloc
            ps_k1 = psKQ.tile([H_DIM, IN_DIM], f32, tag="kq")
            for cn in range(NCH_N):
                for half in range(2):
                    nc.tensor.matmul(
                        ps_k1[:, half * NH:(half + 1) * NH],
                        kw_sb[:, cn, :],
                        s_bf1[:, cn, half * NH:(half + 1) * NH],
                        start=(cn == 0),
                        stop=(cn == NCH_N - 1),
                    )
            k_sb1 = emit_k_evict(ps_k1)
            emit_g_casts()
            q_sb1 = emit_q(s_T1)

            # all of batch1's att1 tiles, then one PAIRED att2 stream
            for idx in range(16):
                ci, half = ATT1_ORDER[idx]
                emit_att1_group(att1sq1, k_sb1, q_sb1, ci, half, idx)

            for nt in range(NCH_N):
                phase_att2_pair(att1sq0, att1sq1, nt)

    nc.compile()
    return nc


def _get_nc(mm_mode="v4"):
    if mm_mode not in _NC_CACHE:
        _NC_CACHE[mm_mode] = _build_nc(mm_mode)
    return _NC_CACHE[mm_mode]


def _run(inputs, trace=False, mm_mode="v4", tmpdir=None):
    from concourse.bass_utils import run_bass_kernel_spmd

    s = np.ascontiguousarray(np.asarray(inputs["s"], dtype=np.float32))
    g = np.ascontiguousarray(np.asarray(inputs["Gmat"], dtype=np.float32))
    qw = np.ascontiguousarray(np.asarray(inputs["Qweight"], dtype=np.float32))
    kw = np.ascontiguousarray(np.asarray(inputs["Kweight"], dtype=np.float32))

    nc = _get_nc("v4")
    in_maps = [
        {
            "s": np.ascontiguousarray(s[:, c * B_LOC:(c + 1) * B_LOC, :]),
            "gmat": g,
            "qw": qw,
            "kw": kw,
        }
        for c in range(N_CORES)
    ]
    res = run_bass_kernel_spmd(
        nc, in_maps, list(range(N_CORES)), trace=trace, tmpdir=tmpdir
    )
    out = np.concatenate(
        [res.results[c]["out"] for c in range(N_CORES)], axis=1
    )
    return out, res


def kernel(**inputs) -> np.ndarray:
    out, _ = _run(inputs, trace=False)
    return out


# revision 63
# speedup vs baseline: 1.1462x; 1.1462x over previous
"""Trainium2 Bass kernel for nn_GAttention (gnn_message_passing).

Reference computation (per batch b):
    q = s[:,b,:] @ Qweight                      # (N, H)
    k = Kweight.T @ s[:,b,:]                    # (H, I)   (contraction over n)
    att1 = (q @ k) * (1/sqrt(H)) + 1e-9         # (N, I)
    att2 = att1**2 @ Gmat                       # (N, I)
    out[:,b,:] = att2 / (rowsum(att2) + 1e-3)

Sharding: pure data-parallel over batch B=16 -> 2 batches per core on 8 cores.
Gmat/Qweight/Kweight replicated.

v4: bf16 front-end + fp8 DoubleRow back-end.
  - att1sq and Gmat are fp8e4 and the dominant att2 GEMM (1.07 GMAC/batch)
    runs double-pumped (MatmulPerfMode.DoubleRow, 2 contraction chunks per
    instruction, 216ns per 512-wide matmul == 157 TF/s). Measured end-to-end
    rel err ~2.7e-3 vs the f32 reference (gate 2e-2): fp8 noise feeds
    all-positive 1024-term sums and the normalization rowsum is accumulated
    from the same quantized products, so num/denom stay consistent.
  - everything upstream (s, s_T, k, q, att1) stays bf16: fp8 CONVERSIONS on
    ACT/DVE measure ~2x slower than bf16 ones (f32->fp8 cast ~1.3us/chunk vs
    0.73, PSUM->fp8 evict 710ns vs 418ns per 512), so fp8 there loses more
    engine time than DoubleRow saves in PE time.
  - normalize: the two att2 halves accumulate into one 2-bank PSUM tile so
    a single 1024-wide ACT eviction with fused accum_out yields the full
    rowsum in one op; rinv is a tiny DVE add+reciprocal and one 1024-wide
    DVE multiply scales the tile in place; each output half DMAs as soon as
    it is scaled. (GPSIMD was tried for the scale: 14.8us per op, ~30x
    slower than DVE, plus SBUF port contention.)
  - HBM is the binding roofline (20.8 MB/core, 358 GB/s peak; outputs
    sustain only ~200 GB/s on 4KB-row strided writes). All input DMAs ride
    the Sync queue set in priority order s(b0) -> s(b1) -> G with NO
    completion-dependency ladders (those serialized input arrival to ~59us).
  - engine-queue discipline: each engine runs its ops in emission order, so
    ops feeding the PE next are never emitted behind ops gated on later
    results (ALL G casts deferred past batch0's A/B phases, kq evictions
    split ACT/DVE to halve their serial latency).
  - lead-in: batch0 chunk0 arrives as four column-quarter DMAs on separate
    queues and is cast/transposed piecewise, so the PE starts ~10us in.
  - att2 group nt only reads att1sq n-columns nt*128:(nt+1)*128 (half
    nt//4), so only half-0 att1 tiles gate each C phase; the others are
    woven into the att2 group streams.

The two batches are software-pipelined: batch1's cast/transpose chunks are
interleaved with batch0's att1 tiles (trailing the s1 DMA stream), its k/q
are built just before C(0), and its att1 tiles are woven into both att2
group streams so the PE always has independent work while PSUM banks drain.
Run-to-run HW variance is ~+/-10% (108-127us observed for this binary).
"""

import sys

import numpy as np

try:  # concourse normally comes from the image's NIX_PYTHONPATH
    import concourse  # noqa: F401
except ImportError:  # pragma: no cover
    sys.path.insert(0, "/opt/trn_rl_repo")

N_DIM = 1024
IN_DIM = 1024
H_DIM = 64
B = 16
N_CORES = 8
B_LOC = B // N_CORES  # batches per core

P = 128          # SBUF/PSUM partitions
NCH_N = N_DIM // P   # 8 chunks over n
NCH_I = IN_DIM // P  # 8 chunks over i
NH = 512         # psum free-dim half (one fp32 bank)
NPAIR = NCH_I // 2   # DoubleRow contracts chunk pairs

_NC_CACHE = {}


def _build_nc(mm_mode="v4"):
    import concourse.bass as bass
    import concourse.tile as tile
    from concourse import bacc, mybir
    from concourse.masks import make_identity

    f32 = mybir.dt.float32
    bf16 = mybir.dt.bfloat16
    fp8 = mybir.dt.float8e4
    AFT = mybir.ActivationFunctionType
    ALU = mybir.AluOpType
    DR = mybir.MatmulPerfMode.DoubleRow

    nc = bacc.Bacc(
        "TRN2",
        target_bir_lowering=False,
        debug=False,
        num_devices=N_CORES,
    )
    s_d = nc.dram_tensor("s", [N_DIM, B_LOC, IN_DIM], f32, kind="ExternalInput")
    g_d = nc.dram_tensor("gmat", [IN_DIM, IN_DIM], f32, kind="ExternalInput")
    qw_d = nc.dram_tensor("qw", [IN_DIM, H_DIM], f32, kind="ExternalInput")
    kw_d = nc.dram_tensor("kw", [N_DIM, H_DIM], f32, kind="ExternalInput")
    o_d = nc.dram_tensor("out", [N_DIM, B_LOC, IN_DIM], f32, kind="ExternalOutput")

    with tile.TileContext(nc) as tc:
        with (
            tc.tile_pool(name="const", bufs=1) as const_pool,
            tc.tile_pool(name="gmat", bufs=1) as gmat_pool,
            tc.tile_pool(name="snat", bufs=1) as snat_pool,
            tc.tile_pool(name="sT", bufs=2) as sT_pool,
            tc.tile_pool(name="att1", bufs=2) as att1_pool,
            tc.tile_pool(name="kq", bufs=1) as kq_pool,
            tc.tile_pool(name="outs", bufs=3) as out_pool,
            tc.tile_pool(name="stage", bufs=2) as stage_pool,
            tc.tile_pool(name="sbf", bufs=2) as sbf_pool,
            tc.tile_pool(name="stat", bufs=8) as stat_pool,
            tc.tile_pool(name="psA", bufs=2, space="PSUM") as psA,
            tc.tile_pool(name="psO", bufs=2, space="PSUM") as psO,
            tc.tile_pool(name="psKQ", bufs=1, space="PSUM") as psKQ,
        ):
            ident_f32 = const_pool.tile([P, P], f32)
            make_identity(nc, ident_f32[:])
            ident_bf = const_pool.tile([P, P], bf16)
            nc.vector.tensor_copy(ident_bf[:], ident_f32[:])

            eps_bias = const_pool.tile([P, 1], f32)
            nc.vector.memset(eps_bias[:], 1e-9)

            # ---- batch0 s chunks first: chunk0 split into column quarters
            # on separate queues so the first cast/transpose starts early.
            def phase_load_s(b, split0):
                s_view = s_d.ap()[:, b, :]
                s_nat = snat_pool.tile([P, NCH_N, IN_DIM], f32, tag="snat")
                dmas = []
                qtrs = None
                for cn in range(NCH_N):
                    if cn == 0 and split0:
                        qtrs = []
                        for qtr in range(4):
                            c0, c1 = qtr * 256, (qtr + 1) * 256
                            dd = nc.sync.dma_start(
                                s_nat[:, 0, c0:c1], s_view[0:P, c0:c1]
                            )
                            qtrs.append(dd)
                        dmas.append(qtrs[-1])
                    else:
                        dd = nc.sync.dma_start(
                            s_nat[:, cn, :], s_view[cn * P:(cn + 1) * P, :]
                        )
                        dmas.append(dd)
                return s_nat, dmas, qtrs

            s_nat0, s_dmas0, s0_qtrs = phase_load_s(0, split0=True)

            # weights (small) after the critical s chunks
            qw_f32 = const_pool.tile([P, NCH_I, H_DIM], f32)
            nc.sync.dma_start(
                qw_f32[:], qw_d.ap().rearrange("(c p) h -> p c h", p=P)
            )
            qw_sb = const_pool.tile([P, NCH_I, H_DIM], bf16)
            nc.vector.tensor_copy(qw_sb[:], qw_f32[:])
            kw_f32 = const_pool.tile([P, NCH_N, H_DIM], f32)
            nc.sync.dma_start(
                kw_f32[:], kw_d.ap().rearrange("(c p) h -> p c h", p=P)
            )
            kw_sb = const_pool.tile([P, NCH_N, H_DIM], bf16)
            nc.vector.tensor_copy(kw_sb[:], kw_f32[:])

            # HBM priority (one Sync queue set, order = priority, no
            # completion ladders): s(b0) -> s(b1) -> G. Batch1's
            # cast/transpose work is s1-arrival-paced and overlaps batch0's
            # B phase; G is needed last (first att2 group). (G-before-s1 was
            # measured too: it starts C(0) ~3us earlier but stalls batch1's
            # pipeline more than it gains.)
            # Gmat staged whole in f32; ALL casts deferred -- inline casts at
            # the head of the ACT/DVE queues would block every s cast behind
            # them while waiting for G chunks to arrive.
            s_nat1, s_dmas1, _ = phase_load_s(1, split0=False)
            g_sb = gmat_pool.tile([P, NCH_I, IN_DIM], fp8)
            g_view = g_d.ap()
            g_f32 = gmat_pool.tile([P, NCH_I, IN_DIM], f32)
            for ci in range(NCH_I):
                nc.sync.dma_start(
                    g_f32[:, ci, :], g_view[ci * P:(ci + 1) * P, :]
                )

            def emit_g_casts():
                for ci in range(NCH_I):
                    if ci % 2 == 0:
                        nc.scalar.activation(
                            g_sb[:, ci, :], g_f32[:, ci, :], AFT.Copy
                        )
                    else:
                        nc.vector.tensor_copy(
                            g_sb[:, ci, :], g_f32[:, ci, :]
                        )

            def cast_chunk(s_nat, s_bf, cn, qtr=None, on_dve=False):
                # batch0 casts on ACT (free during A(0)); batch1 casts on
                # DVE -- the ACT queue at that time holds the kq evictions
                # and att1 squares, which wait on q0 and would head-of-line
                # block casts whose data is already resident.
                if qtr is None:
                    if on_dve:
                        nc.vector.tensor_copy(s_bf[:, cn, :], s_nat[:, cn, :])
                    else:
                        nc.scalar.activation(
                            s_bf[:, cn, :], s_nat[:, cn, :], AFT.Copy
                        )
                else:
                    c0, c1 = qtr * 256, (qtr + 1) * 256
                    if qtr % 2 == 0:
                        nc.scalar.activation(
                            s_bf[:, cn, c0:c1], s_nat[:, cn, c0:c1], AFT.Copy
                        )
                    else:
                        nc.vector.tensor_copy(
                            s_bf[:, cn, c0:c1], s_nat[:, cn, c0:c1]
                        )

            def transposes_cig(s_bf, s_T, cn, cig):
                """4 transpose blocks (128x128), transpose-mode so the PSUM
                tile stays bf16 and the eviction is a cheap same-dtype COPY
                (418 vs 690 ns per 512)."""
                pt = psA.tile([P, NH], bf16, tag="ps512")
                for blk in range(4):
                    ci = cig * 4 + blk
                    nc.tensor.transpose(
                        pt[:, blk * P:(blk + 1) * P],
                        s_bf[:, cn, ci * P:(ci + 1) * P],
                        ident_bf[:],
                    )
                nc.vector.tensor_copy(
                    s_T[:, cig * 4:(cig + 1) * 4, cn * P:(cn + 1) * P],
                    pt[:].rearrange("p (c n) -> p c n", c=4),
                )

            def phase_tk_chunk(s_nat, s_bf, s_T, ps_k, cn, first_split=False,
                               on_dve=False):
                """Cast + transposes + k-matmul contribution for one chunk."""
                if cn == 0 and first_split:
                    cast_chunk(s_nat, s_bf, 0, qtr=0)
                    cast_chunk(s_nat, s_bf, 0, qtr=1)
                    transposes_cig(s_bf, s_T, 0, 0)
                    cast_chunk(s_nat, s_bf, 0, qtr=2)
                    cast_chunk(s_nat, s_bf, 0, qtr=3)
                    transposes_cig(s_bf, s_T, 0, 1)
                else:
                    cast_chunk(s_nat, s_bf, cn, on_dve=on_dve)
                    for cig in range(2):
                        transposes_cig(s_bf, s_T, cn, cig)
                for half in range(2):
                    nc.tensor.matmul(
                        ps_k[:, half * NH:(half + 1) * NH],
                        kw_sb[:, cn, :],
                        s_bf[:, cn, half * NH:(half + 1) * NH],
                        start=(cn == 0),
                        stop=(cn == NCH_N - 1),
                    )

            def emit_k_evict(ps_k):
                # halves evicted on ACT and DVE concurrently: this sits on
                # the serial chain k-psum -> att1, so latency matters
                k_sb = kq_pool.tile([H_DIM, IN_DIM], bf16, tag="k")
                nc.scalar.activation(k_sb[:, 0:NH], ps_k[:, 0:NH], AFT.Copy)
                nc.vector.tensor_copy(k_sb[:, NH:2 * NH], ps_k[:, NH:2 * NH])
                return k_sb

            def emit_q(s_T):
                ps_q = psKQ.tile([H_DIM, N_DIM], f32, tag="kq")
                for ci in range(NCH_I):
                    for half in range(2):
                        nc.tensor.matmul(
                            ps_q[:, half * NH:(half + 1) * NH],
                            qw_sb[:, ci, :],
                            s_T[:, ci, half * NH:(half + 1) * NH],
                            start=(ci == 0),
                            stop=(ci == NCH_I - 1),
                        )
                q_sb = kq_pool.tile([H_DIM, N_DIM], bf16, tag="q")
                nc.scalar.activation(q_sb[:, 0:NH], ps_q[:, 0:NH], AFT.Copy)
                nc.vector.tensor_copy(q_sb[:, NH:2 * NH], ps_q[:, NH:2 * NH])
                return q_sb

            def emit_att1_group(att1sq, k_sb, q_sb, ci, half, idx):
                """att1T tile (ci, half): K=64 bf16 matmul then fused
                square+scale into fp8. Mostly ACT; some tiles on DVE."""
                pa = psA.tile([P, NH], f32, tag="ps512")
                nc.tensor.matmul(
                    pa[:],
                    k_sb[:, ci * P:(ci + 1) * P],
                    q_sb[:, half * NH:(half + 1) * NH],
                    start=True,
                    stop=True,
                )
                dst = att1sq[:, ci, half * NH:(half + 1) * NH]
                if idx % 3 != 2:
                    nc.scalar.activation(
                        dst, pa[:], AFT.Square, bias=eps_bias[:], scale=0.125
                    )
                else:
                    # DVE path needs 2 ops (hw allows only one PSUM operand);
                    # bf16 tmp gets 2x DVE rate on the second op.
                    tmp = stage_pool.tile([P, NH], bf16, tag="sqtmp")
                    nc.vector.tensor_scalar(
                        tmp[:], pa[:], 0.125, 1e-9,
                        op0=ALU.mult, op1=ALU.add,
                    )
                    nc.vector.tensor_mul(dst, tmp[:], tmp[:])

            def phase_att2_group(b, att1sq, nt):
                """One att2 output tile. The two 512-halves accumulate into
                one 2-bank PSUM tile so a single 1024-wide ACT eviction with
                fused accum_out yields the FULL rowsum in one op (1024-wide
                ACT runs at 0.9 elem/ns vs 0.75 for 512-wide, and the
                rs0+rs1 combine disappears)."""
                po = psO.tile([P, 2 * NH], f32, tag="psO")
                for c in range(NPAIR):
                    lhsT = att1sq[:, 2 * c:2 * c + 2, nt * P:(nt + 1) * P]
                    nc.tensor.matmul(
                        po[:, 0:NH], lhsT, g_sb[:, 2 * c:2 * c + 2, 0:NH],
                        start=(c == 0), stop=(c == NPAIR - 1),
                        perf_mode=DR,
                    )
                    nc.tensor.matmul(
                        po[:, NH:2 * NH], lhsT, g_sb[:, 2 * c:2 * c + 2, NH:2 * NH],
                        start=(c == 0), stop=(c == NPAIR - 1),
                        perf_mode=DR,
                    )
                ot = out_pool.tile([P, IN_DIM], f32, tag="out")
                rs = stat_pool.tile([P, 1], f32, tag="rs")
                nc.scalar.activation(
                    ot[:], po[:], AFT.Copy, accum_out=rs[:]
                )
                rinv = stat_pool.tile([P, 1], f32, tag="rinv")
                nc.vector.tensor_scalar_add(rinv[:], rs[:], 1e-3)
                nc.vector.reciprocal(rinv[:], rinv[:])
                nc.vector.tensor_scalar_mul(ot[:], ot[:], rinv[:])
                # one full-tile DMA with 4KB contiguous rows: column-split
                # halves (2KB rows) measured ~2x lower DRAM write throughput,
                # and row-split halves bought nothing (the drain is aggregate
                # write-bandwidth-bound, not per-transfer-bound).
                return nc.sync.dma_start(
                    o_d.ap()[nt * P:(nt + 1) * P, b, :], ot[:]
                )

            # ---- software pipeline over the two batches. HBM is the
            # binding roofline, so the DMA stream (s0, s1, G, outputs) is
            # front-loaded and compute trails it:
            #   A(0): per-chunk cast+transpose+k as s0 chunks land
            #   kq(0), then att1(0)-half0 INTERLEAVED with batch1's
            #   cast/transpose chunks (which trail the s1 DMA stream),
            #   att1(0)-half1, k(1), G casts, q(1),
            #   C(0) weaving att1(1)-half0, C(1) weaving att1(1)-half1.
            ATT1_ORDER = [(ci, half) for half in range(2) for ci in range(NCH_I)]

            s_bf0 = sbf_pool.tile([P, NCH_N, IN_DIM], bf16, tag="sbf")
            s_T0 = sT_pool.tile([P, NCH_I, N_DIM], bf16, tag="sT")
            ps_k0 = psKQ.tile([H_DIM, IN_DIM], f32, tag="kq")
            for cn in range(NCH_N):
                phase_tk_chunk(s_nat0, s_bf0, s_T0, ps_k0, cn,
                               first_split=(cn == 0))

            k_sb0 = emit_k_evict(ps_k0)
            q_sb0 = emit_q(s_T0)

            s_bf1 = sbf_pool.tile([P, NCH_N, IN_DIM], bf16, tag="sbf")
            s_T1 = sT_pool.tile([P, NCH_I, N_DIM], bf16, tag="sT")
            att1sq0 = att1_pool.tile([P, NCH_I, N_DIM], fp8, tag="att1")
            att1sq1 = att1_pool.tile([P, NCH_I, N_DIM], fp8, tag="att1")

            # batch0's att1 half-0 tiles interleaved with batch1's
            # cast/transpose chunks (which trail the s1 DMA stream), then
            # batch0's half-1 tiles.
            for idx in range(8):
                ci, half = ATT1_ORDER[idx]
                emit_att1_group(att1sq0, k_sb0, q_sb0, ci, half, idx)
                # batch1 casts on DVE: on ACT they would queue behind the
                # q0-gated att1 squares even though their data is resident
                cast_chunk(s_nat1, s_bf1, idx, on_dve=True)
                for cig in range(2):
                    transposes_cig(s_bf1, s_T1, idx, cig)
            for idx in range(8, 16):
                ci, half = ATT1_ORDER[idx]
                emit_att1_group(att1sq0, k_sb0, q_sb0, ci, half, idx)

            # batch1 k: all chunks are resident by now, accumulate en bloc
            ps_k1 = psKQ.tile([H_DIM, IN_DIM], f32, tag="kq")
            for cn in range(NCH_N):
                for half in range(2):
                    nc.tensor.matmul(
                        ps_k1[:, half * NH:(half + 1) * NH],
                        kw_sb[:, cn, :],
                        s_bf1[:, cn, half * NH:(half + 1) * NH],
                        start=(cn == 0),
                        stop=(cn == NCH_N - 1),
                    )
            k_sb1 = emit_k_evict(ps_k1)
            emit_g_casts()
            q_sb1 = emit_q(s_T1)

            # C(0) weaving b1's half-0 att1 tiles into groups 4-7 ONLY: a
            # woven tile's matmul sits in the PE's in-order stream, so a tile
            # emitted at group 0 would gate ALL later att2 groups on q1
            # (observed as a ~19us stall). By group 4 q1 is long ready.
            for nt in range(NCH_N):
                phase_att2_group(0, att1sq0, nt)
                if nt >= 4:
                    for idx in (2 * (nt - 4), 2 * (nt - 4) + 1):
                        ci, half = ATT1_ORDER[idx]
                        emit_att1_group(att1sq1, k_sb1, q_sb1, ci, half, idx)

            # C(1) weaving b1's half-1 att1 tiles (needed from group 4 on)
            for nt in range(NCH_N):
                phase_att2_group(1, att1sq1, nt)
                if nt < 4:
                    for idx in (8 + 2 * nt, 9 + 2 * nt):
                        ci, half = ATT1_ORDER[idx]
                        emit_att1_group(att1sq1, k_sb1, q_sb1, ci, half, idx)

    nc.compile()
    return nc


def _get_nc(mm_mode="v4"):
    if mm_mode not in _NC_CACHE:
        _NC_CACHE[mm_mode] = _build_nc(mm_mode)
    return _NC_CACHE[mm_mode]


def _run(inputs, trace=False, mm_mode="v4", tmpdir=None):
    from concourse.bass_utils import run_bass_kernel_spmd

    s = np.ascontiguousarray(np.asarray(inputs["s"], dtype=np.float32))
    g = np.ascontiguousarray(np.asarray(inputs["Gmat"], dtype=np.float32))
    qw = np.ascontiguousarray(np.asarray(inputs["Qweight"], dtype=np.float32))
    kw = np.ascontiguousarray(np.asarray(inputs["Kweight"], dtype=np.float32))

    nc = _get_nc("v4")
    in_maps = [
        {
            "s": np.ascontiguousarray(s[:, c * B_LOC:(c + 1) * B_LOC, :]),
            "gmat": g,
            "qw": qw,
            "kw": kw,
        }
        for c in range(N_CORES)
    ]
    res = run_bass_kernel_spmd(
        nc, in_maps, list(range(N_CORES)), trace=trace, tmpdir=tmpdir
    )
    out = np.concatenate(
        [res.results[c]["out"] for c in range(N_CORES)], axis=1
    )
    return out, res


def kernel(**inputs) -> np.ndarray:
    out, _ = _run(inputs, trace=False)
    return out


# revision 64
# speedup vs baseline: 1.2206x; 1.0649x over previous
"""Trainium2 Bass kernel for nn_GAttention (gnn_message_passing).

Reference computation (per batch b):
    q = s[:,b,:] @ Qweight                      # (N, H)
    k = Kweight.T @ s[:,b,:]                    # (H, I)   (contraction over n)
    att1 = (q @ k) * (1/sqrt(H)) + 1e-9         # (N, I)
    att2 = att1**2 @ Gmat                       # (N, I)
    out[:,b,:] = att2 / (rowsum(att2) + 1e-3)

Sharding: pure data-parallel over batch B=16 -> 2 batches per core on 8 cores.
Gmat/Qweight/Kweight replicated.

v4: bf16 front-end + fp8 DoubleRow back-end.
  - att1sq and Gmat are fp8e4 and the dominant att2 GEMM (1.07 GMAC/batch)
    runs double-pumped (MatmulPerfMode.DoubleRow, 2 contraction chunks per
    instruction, 216ns per 512-wide matmul == 157 TF/s). Measured end-to-end
    rel err ~2.7e-3 vs the f32 reference (gate 2e-2): fp8 noise feeds
    all-positive 1024-term sums and the normalization rowsum is accumulated
    from the same quantized products, so num/denom stay consistent.
  - everything upstream (s, s_T, k, q, att1) stays bf16: fp8 CONVERSIONS on
    ACT/DVE measure ~2x slower than bf16 ones (f32->fp8 cast ~1.3us/chunk vs
    0.73, PSUM->fp8 evict 710ns vs 418ns per 512), so fp8 there loses more
    engine time than DoubleRow saves in PE time.
  - normalize: the two att2 halves accumulate into one 2-bank PSUM tile so
    a single 1024-wide ACT eviction with fused accum_out yields the full
    rowsum in one op; rinv is a tiny DVE add+reciprocal and one 1024-wide
    DVE multiply scales the tile in place; each output half DMAs as soon as
    it is scaled. (GPSIMD was tried for the scale: 14.8us per op, ~30x
    slower than DVE, plus SBUF port contention.)
  - HBM is the binding roofline (20.8 MB/core, 358 GB/s peak; outputs
    sustain only ~200 GB/s on 4KB-row strided writes). All input DMAs ride
    the Sync queue set in priority order s(b0) -> s(b1) -> G with NO
    completion-dependency ladders (those serialized input arrival to ~59us).
  - engine-queue discipline: each engine runs its ops in emission order, so
    ops feeding the PE next are never emitted behind ops gated on later
    results (ALL G casts deferred past batch0's A/B phases, kq evictions
    split ACT/DVE to halve their serial latency).
  - lead-in: batch0 chunk0 arrives as four column-quarter DMAs on separate
    queues and is cast/transposed piecewise, so the PE starts ~10us in.
  - att2 group nt only reads att1sq n-columns nt*128:(nt+1)*128 (half
    nt//4), so only half-0 att1 tiles gate each C phase; the others are
    woven into the att2 group streams.

The two batches are software-pipelined: batch1's cast/transpose chunks are
interleaved with batch0's att1 tiles (trailing the s1 DMA stream), its k/q
are built just before C(0), and its att1 tiles are woven into both att2
group streams so the PE always has independent work while PSUM banks drain.
Run-to-run HW variance is ~+/-10% (108-127us observed for this binary).
"""

import sys

import numpy as np

try:  # concourse normally comes from the image's NIX_PYTHONPATH
    import concourse  # noqa: F401
except ImportError:  # pragma: no cover
    sys.path.insert(0, "/opt/trn_rl_repo")

N_DIM = 1024
IN_DIM = 1024
H_DIM = 64
B = 16
N_CORES = 8
B_LOC = B // N_CORES  # batches per core

P = 128          # SBUF/PSUM partitions
NCH_N = N_DIM // P   # 8 chunks over n
NCH_I = IN_DIM // P  # 8 chunks over i
NH = 512         # psum free-dim half (one fp32 bank)
NPAIR = NCH_I // 2   # DoubleRow contracts chunk pairs

_NC_CACHE = {}


def _build_nc(mm_mode="v4"):
    import concourse.bass as bass
    import concourse.tile as tile
    from concourse import bacc, mybir
    from concourse.masks import make_identity

    f32 = mybir.dt.float32
    bf16 = mybir.dt.bfloat16
    fp8 = mybir.dt.float8e4
    AFT = mybir.ActivationFunctionType
    ALU = mybir.AluOpType
    DR = mybir.MatmulPerfMode.DoubleRow

    nc = bacc.Bacc(
        "TRN2",
        target_bir_lowering=False,
        debug=False,
        num_devices=N_CORES,
    )
    s_d = nc.dram_tensor("s", [N_DIM, B_LOC, IN_DIM], f32, kind="ExternalInput")
    g_d = nc.dram_tensor("gmat", [IN_DIM, IN_DIM], f32, kind="ExternalInput")
    qw_d = nc.dram_tensor("qw", [IN_DIM, H_DIM], f32, kind="ExternalInput")
    kw_d = nc.dram_tensor("kw", [N_DIM, H_DIM], f32, kind="ExternalInput")
    o_d = nc.dram_tensor("out", [N_DIM, B_LOC, IN_DIM], f32, kind="ExternalOutput")

    with tile.TileContext(nc) as tc:
        with (
            tc.tile_pool(name="const", bufs=1) as const_pool,
            tc.tile_pool(name="gmat", bufs=1) as gmat_pool,
            tc.tile_pool(name="snat", bufs=1) as snat_pool,
            tc.tile_pool(name="sT", bufs=2) as sT_pool,
            tc.tile_pool(name="att1", bufs=2) as att1_pool,
            tc.tile_pool(name="kq", bufs=1) as kq_pool,
            tc.tile_pool(name="outs", bufs=4) as out_pool,
            tc.tile_pool(name="stage", bufs=2) as stage_pool,
            tc.tile_pool(name="sbf", bufs=2) as sbf_pool,
            tc.tile_pool(name="stat", bufs=8) as stat_pool,
            tc.tile_pool(name="psA", bufs=2, space="PSUM") as psA,
            tc.tile_pool(name="psO", bufs=2, space="PSUM") as psO,
            tc.tile_pool(name="psKQ", bufs=1, space="PSUM") as psKQ,
        ):
            ident_f32 = const_pool.tile([P, P], f32)
            make_identity(nc, ident_f32[:])
            ident_bf = const_pool.tile([P, P], bf16)
            nc.vector.tensor_copy(ident_bf[:], ident_f32[:])

            eps_bias = const_pool.tile([P, 1], f32)
            nc.vector.memset(eps_bias[:], 1e-9)

            # ---- batch0 s chunks first: chunk0 split into column quarters
            # on separate queues so the first cast/transpose starts early.
            def phase_load_s(b, split0):
                s_view = s_d.ap()[:, b, :]
                s_nat = snat_pool.tile([P, NCH_N, IN_DIM], f32, tag="snat")
                dmas = []
                qtrs = None
                for cn in range(NCH_N):
                    if cn == 0 and split0:
                        qtrs = []
                        for qtr in range(4):
                            c0, c1 = qtr * 256, (qtr + 1) * 256
                            dd = nc.sync.dma_start(
                                s_nat[:, 0, c0:c1], s_view[0:P, c0:c1]
                            )
                            qtrs.append(dd)
                        dmas.append(qtrs[-1])
                    else:
                        dd = nc.sync.dma_start(
                            s_nat[:, cn, :], s_view[cn * P:(cn + 1) * P, :]
                        )
                        dmas.append(dd)
                return s_nat, dmas, qtrs

            s_nat0, s_dmas0, s0_qtrs = phase_load_s(0, split0=True)

            # weights (small) after the critical s chunks
            qw_f32 = const_pool.tile([P, NCH_I, H_DIM], f32)
            nc.sync.dma_start(
                qw_f32[:], qw_d.ap().rearrange("(c p) h -> p c h", p=P)
            )
            qw_sb = const_pool.tile([P, NCH_I, H_DIM], bf16)
            nc.vector.tensor_copy(qw_sb[:], qw_f32[:])
            kw_f32 = const_pool.tile([P, NCH_N, H_DIM], f32)
            nc.sync.dma_start(
                kw_f32[:], kw_d.ap().rearrange("(c p) h -> p c h", p=P)
            )
            kw_sb = const_pool.tile([P, NCH_N, H_DIM], bf16)
            nc.vector.tensor_copy(kw_sb[:], kw_f32[:])

            # HBM priority (one Sync queue set, order = priority, no
            # completion ladders): s(b0) -> s(b1) -> G. Batch1's
            # cast/transpose work is s1-arrival-paced and overlaps batch0's
            # B phase; G is needed last (first att2 group). (G-before-s1 was
            # measured too: it starts C(0) ~3us earlier but stalls batch1's
            # pipeline more than it gains.)
            # Gmat staged whole in f32; ALL casts deferred -- inline casts at
            # the head of the ACT/DVE queues would block every s cast behind
            # them while waiting for G chunks to arrive.
            s_nat1, s_dmas1, _ = phase_load_s(1, split0=False)
            g_sb = gmat_pool.tile([P, NCH_I, IN_DIM], fp8)
            g_view = g_d.ap()
            g_f32 = gmat_pool.tile([P, NCH_I, IN_DIM], f32)
            for ci in range(NCH_I):
                nc.sync.dma_start(
                    g_f32[:, ci, :], g_view[ci * P:(ci + 1) * P, :]
                )

            def emit_g_casts():
                for ci in range(NCH_I):
                    if ci % 2 == 0:
                        nc.scalar.activation(
                            g_sb[:, ci, :], g_f32[:, ci, :], AFT.Copy
                        )
                    else:
                        nc.vector.tensor_copy(
                            g_sb[:, ci, :], g_f32[:, ci, :]
                        )

            def cast_chunk(s_nat, s_bf, cn, qtr=None, on_dve=False):
                # batch0 casts on ACT (free during A(0)); batch1 casts on
                # DVE -- the ACT queue at that time holds the kq evictions
                # and att1 squares, which wait on q0 and would head-of-line
                # block casts whose data is already resident.
                if qtr is None:
                    if on_dve:
                        nc.vector.tensor_copy(s_bf[:, cn, :], s_nat[:, cn, :])
                    else:
                        nc.scalar.activation(
                            s_bf[:, cn, :], s_nat[:, cn, :], AFT.Copy
                        )
                else:
                    c0, c1 = qtr * 256, (qtr + 1) * 256
                    if qtr % 2 == 0:
                        nc.scalar.activation(
                            s_bf[:, cn, c0:c1], s_nat[:, cn, c0:c1], AFT.Copy
                        )
                    else:
                        nc.vector.tensor_copy(
                            s_bf[:, cn, c0:c1], s_nat[:, cn, c0:c1]
                        )

            def transposes_cig(s_bf, s_T, cn, cig):
                """4 transpose blocks (128x128), transpose-mode so the PSUM
                tile stays bf16 and the eviction is a cheap same-dtype COPY
                (418 vs 690 ns per 512)."""
                pt = psA.tile([P, NH], bf16, tag="ps512")
                for blk in range(4):
                    ci = cig * 4 + blk
                    nc.tensor.transpose(
                        pt[:, blk * P:(blk + 1) * P],
                        s_bf[:, cn, ci * P:(ci + 1) * P],
                        ident_bf[:],
                    )
                nc.vector.tensor_copy(
                    s_T[:, cig * 4:(cig + 1) * 4, cn * P:(cn + 1) * P],
                    pt[:].rearrange("p (c n) -> p c n", c=4),
                )

            def phase_tk_chunk(s_nat, s_bf, s_T, ps_k, cn, first_split=False,
                               on_dve=False):
                """Cast + transposes + k-matmul contribution for one chunk."""
                if cn == 0 and first_split:
                    cast_chunk(s_nat, s_bf, 0, qtr=0)
                    cast_chunk(s_nat, s_bf, 0, qtr=1)
                    transposes_cig(s_bf, s_T, 0, 0)
                    cast_chunk(s_nat, s_bf, 0, qtr=2)
                    cast_chunk(s_nat, s_bf, 0, qtr=3)
                    transposes_cig(s_bf, s_T, 0, 1)
                else:
                    cast_chunk(s_nat, s_bf, cn, on_dve=on_dve)
                    for cig in range(2):
                        transposes_cig(s_bf, s_T, cn, cig)
                for half in range(2):
                    nc.tensor.matmul(
                        ps_k[:, half * NH:(half + 1) * NH],
                        kw_sb[:, cn, :],
                        s_bf[:, cn, half * NH:(half + 1) * NH],
                        start=(cn == 0),
                        stop=(cn == NCH_N - 1),
                    )

            def emit_k_evict(ps_k):
                # halves evicted on ACT and DVE concurrently: this sits on
                # the serial chain k-psum -> att1, so latency matters
                k_sb = kq_pool.tile([H_DIM, IN_DIM], bf16, tag="k")
                nc.scalar.activation(k_sb[:, 0:NH], ps_k[:, 0:NH], AFT.Copy)
                nc.vector.tensor_copy(k_sb[:, NH:2 * NH], ps_k[:, NH:2 * NH])
                return k_sb

            def emit_q(s_T):
                ps_q = psKQ.tile([H_DIM, N_DIM], f32, tag="kq")
                for ci in range(NCH_I):
                    for half in range(2):
                        nc.tensor.matmul(
                            ps_q[:, half * NH:(half + 1) * NH],
                            qw_sb[:, ci, :],
                            s_T[:, ci, half * NH:(half + 1) * NH],
                            start=(ci == 0),
                            stop=(ci == NCH_I - 1),
                        )
                q_sb = kq_pool.tile([H_DIM, N_DIM], bf16, tag="q")
                nc.scalar.activation(q_sb[:, 0:NH], ps_q[:, 0:NH], AFT.Copy)
                nc.vector.tensor_copy(q_sb[:, NH:2 * NH], ps_q[:, NH:2 * NH])
                return q_sb

            def emit_att1_group(att1sq, k_sb, q_sb, ci, half, idx):
                """att1T tile (ci, half): K=64 bf16 matmul then fused
                square+scale into fp8. Mostly ACT; some tiles on DVE."""
                pa = psA.tile([P, NH], f32, tag="ps512")
                nc.tensor.matmul(
                    pa[:],
                    k_sb[:, ci * P:(ci + 1) * P],
                    q_sb[:, half * NH:(half + 1) * NH],
                    start=True,
                    stop=True,
                )
                dst = att1sq[:, ci, half * NH:(half + 1) * NH]
                if idx % 3 != 2:
                    nc.scalar.activation(
                        dst, pa[:], AFT.Square, bias=eps_bias[:], scale=0.125
                    )
                else:
                    # DVE path needs 2 ops (hw allows only one PSUM operand);
                    # bf16 tmp gets 2x DVE rate on the second op.
                    tmp = stage_pool.tile([P, NH], bf16, tag="sqtmp")
                    nc.vector.tensor_scalar(
                        tmp[:], pa[:], 0.125, 1e-9,
                        op0=ALU.mult, op1=ALU.add,
                    )
                    nc.vector.tensor_mul(dst, tmp[:], tmp[:])

            def phase_att2_group(b, att1sq, nt):
                """One att2 output tile. The two 512-halves accumulate into
                one 2-bank PSUM tile so a single 1024-wide ACT eviction with
                fused accum_out yields the FULL rowsum in one op (1024-wide
                ACT runs at 0.9 elem/ns vs 0.75 for 512-wide, and the
                rs0+rs1 combine disappears)."""
                po = psO.tile([P, 2 * NH], f32, tag="psO")
                for c in range(NPAIR):
                    lhsT = att1sq[:, 2 * c:2 * c + 2, nt * P:(nt + 1) * P]
                    nc.tensor.matmul(
                        po[:, 0:NH], lhsT, g_sb[:, 2 * c:2 * c + 2, 0:NH],
                        start=(c == 0), stop=(c == NPAIR - 1),
                        perf_mode=DR,
                    )
                    nc.tensor.matmul(
                        po[:, NH:2 * NH], lhsT, g_sb[:, 2 * c:2 * c + 2, NH:2 * NH],
                        start=(c == 0), stop=(c == NPAIR - 1),
                        perf_mode=DR,
                    )
                ot = out_pool.tile([P, IN_DIM], f32, tag="out")
                rs = stat_pool.tile([P, 1], f32, tag="rs")
                nc.scalar.activation(
                    ot[:], po[:], AFT.Copy, accum_out=rs[:]
                )
                rinv = stat_pool.tile([P, 1], f32, tag="rinv")
                nc.vector.tensor_scalar_add(rinv[:], rs[:], 1e-3)
                nc.vector.reciprocal(rinv[:], rinv[:])
                nc.vector.tensor_scalar_mul(ot[:], ot[:], rinv[:])
                # one full-tile DMA with 4KB contiguous rows: column-split
                # halves (2KB rows) measured ~2x lower DRAM write throughput,
                # and row-split halves bought nothing (the drain is aggregate
                # write-bandwidth-bound, not per-transfer-bound).
                return nc.sync.dma_start(
                    o_d.ap()[nt * P:(nt + 1) * P, b, :], ot[:]
                )

            # ---- software pipeline over the two batches. HBM is the
            # binding roofline, so the DMA stream (s0, s1, G, outputs) is
            # front-loaded and compute trails it:
            #   A(0): per-chunk cast+transpose+k as s0 chunks land
            #   kq(0), then att1(0)-half0 INTERLEAVED with batch1's
            #   cast/transpose chunks (which trail the s1 DMA stream),
            #   att1(0)-half1, k(1), G casts, q(1),
            #   C(0) weaving att1(1)-half0, C(1) weaving att1(1)-half1.
            ATT1_ORDER = [(ci, half) for half in range(2) for ci in range(NCH_I)]

            s_bf0 = sbf_pool.tile([P, NCH_N, IN_DIM], bf16, tag="sbf")
            s_T0 = sT_pool.tile([P, NCH_I, N_DIM], bf16, tag="sT")
            ps_k0 = psKQ.tile([H_DIM, IN_DIM], f32, tag="kq")
            for cn in range(NCH_N):
                phase_tk_chunk(s_nat0, s_bf0, s_T0, ps_k0, cn,
                               first_split=(cn == 0))

            k_sb0 = emit_k_evict(ps_k0)
            q_sb0 = emit_q(s_T0)

            s_bf1 = sbf_pool.tile([P, NCH_N, IN_DIM], bf16, tag="sbf")
            s_T1 = sT_pool.tile([P, NCH_I, N_DIM], bf16, tag="sT")
            att1sq0 = att1_pool.tile([P, NCH_I, N_DIM], fp8, tag="att1")
            att1sq1 = att1_pool.tile([P, NCH_I, N_DIM], fp8, tag="att1")

            # batch0's att1 half-0 tiles interleaved with batch1's
            # cast/transpose chunks (which trail the s1 DMA stream), then
            # batch0's half-1 tiles.
            for idx in range(8):
                ci, half = ATT1_ORDER[idx]
                emit_att1_group(att1sq0, k_sb0, q_sb0, ci, half, idx)
                # batch1 casts on DVE: on ACT they would queue behind the
                # q0-gated att1 squares even though their data is resident
                cast_chunk(s_nat1, s_bf1, idx, on_dve=True)
                for cig in range(2):
                    transposes_cig(s_bf1, s_T1, idx, cig)
            for idx in range(8, 16):
                ci, half = ATT1_ORDER[idx]
                emit_att1_group(att1sq0, k_sb0, q_sb0, ci, half, idx)

            # batch1 k: all chunks are resident by now, accumulate en bloc
            ps_k1 = psKQ.tile([H_DIM, IN_DIM], f32, tag="kq")
            for cn in range(NCH_N):
                for half in range(2):
                    nc.tensor.matmul(
                        ps_k1[:, half * NH:(half + 1) * NH],
                        kw_sb[:, cn, :],
                        s_bf1[:, cn, half * NH:(half + 1) * NH],
                        start=(cn == 0),
                        stop=(cn == NCH_N - 1),
                    )
            k_sb1 = emit_k_evict(ps_k1)
            emit_g_casts()
            q_sb1 = emit_q(s_T1)

            # C(0) weaving b1's half-0 att1 tiles into groups 4-7 ONLY: a
            # woven tile's matmul sits in the PE's in-order stream, so a tile
            # emitted at group 0 would gate ALL later att2 groups on q1
            # (observed as a ~19us stall). By group 4 q1 is long ready.
            for nt in range(NCH_N):
                phase_att2_group(0, att1sq0, nt)
                if nt >= 4:
                    for idx in (2 * (nt - 4), 2 * (nt - 4) + 1):
                        ci, half = ATT1_ORDER[idx]
                        emit_att1_group(att1sq1, k_sb1, q_sb1, ci, half, idx)

            # C(1) weaving b1's half-1 att1 tiles (needed from group 4 on)
            for nt in range(NCH_N):
                phase_att2_group(1, att1sq1, nt)
                if nt < 4:
                    for idx in (8 + 2 * nt, 9 + 2 * nt):
                        ci, half = ATT1_ORDER[idx]
                        emit_att1_group(att1sq1, k_sb1, q_sb1, ci, half, idx)

    nc.compile()
    return nc


def _get_nc(mm_mode="v4"):
    if mm_mode not in _NC_CACHE:
        _NC_CACHE[mm_mode] = _build_nc(mm_mode)
    return _NC_CACHE[mm_mode]


def _run(inputs, trace=False, mm_mode="v4", tmpdir=None):
    from concourse.bass_utils import run_bass_kernel_spmd

    s = np.ascontiguousarray(np.asarray(inputs["s"], dtype=np.float32))
    g = np.ascontiguousarray(np.asarray(inputs["Gmat"], dtype=np.float32))
    qw = np.ascontiguousarray(np.asarray(inputs["Qweight"], dtype=np.float32))
    kw = np.ascontiguousarray(np.asarray(inputs["Kweight"], dtype=np.float32))

    nc = _get_nc("v4")
    in_maps = [
        {
            "s": np.ascontiguousarray(s[:, c * B_LOC:(c + 1) * B_LOC, :]),
            "gmat": g,
            "qw": qw,
            "kw": kw,
        }
        for c in range(N_CORES)
    ]
    res = run_bass_kernel_spmd(
        nc, in_maps, list(range(N_CORES)), trace=trace, tmpdir=tmpdir
    )
    out = np.concatenate(
        [res.results[c]["out"] for c in range(N_CORES)], axis=1
    )
    return out, res


def kernel(**inputs) -> np.ndarray:
    out, _ = _run(inputs, trace=False)
    return out
